# revision 1
# baseline (speedup 1.0000x reference)
"""Trainium2 Bass kernel for causal multi-head attention (B=2, S=2048, E=1024, H=16).

Sharding: 8 cores = 2 batches x 4 head-groups (4 heads each).
Each core computes its batch's QKV for its 4 heads, causal attention, and a
partial output projection; host sums the 4 group partials per batch + b_out.

Optimizations over the first working version (292us -> ~215us):
- matmul inputs DMA'd directly as float32r (no round copies, no staging)
- softmax denominators staged as Ln on the scalar engine; 1/d is a single
  Exp(-x), killing the DVE reciprocal; activation-table fixpoint pinned to
  the combined ln+exp table so no ACT_TABLE_LOAD thrash
- the normalize broadcast is a PE outer product with a selector constant
  (no descriptor-heavy SBUF broadcast DMAs), staged PSUM->SBUF once
- all rollout work after the reciprocal is deferred into the next phase's
  filler stream so neither the PE nor the vector queue stalls at chunk
  boundaries; PV emission runs 6 tiles behind scores for slack
- out-projections scheduled late (chunk 3 is scalar-bound, they keep the
  PE fed); per-phase filler weave balances PE vs scalar per chunk
- PE p-state warm-up dummies during the initial HBM load.
"""
import sys

sys.path.insert(0, "/opt/trn_rl_repo")

from contextlib import ExitStack

import numpy as np

import concourse.bass as bass
import concourse.tile as tile
from concourse import bacc, mybir
from concourse.bass_utils import run_bass_kernel_spmd

dt = mybir.dt

B, S, E, H = 2, 2048, 1024, 16
HD = 64                     # head dim
HPC = 4                     # heads per core
NC = 8                      # cores
KE = E // 128               # 8 contraction k-tiles for projections
NT = S // 128               # 16 token tiles
NCH = S // 512              # 4 token chunks
FQK = 512                   # q+k features per core (4 heads * 64 * 2)
FV = 256                    # v features per core


class _Bacc(bacc.Bacc):
    """Pin the activation-table fixpoint to the combined ln+exp table so the
    Ln (softmax denominator) / Exp mix never thrashes ACT_TABLE_LOADs.
    Table ids are list positions, so other entries stay but turn empty."""

    def insert_act_table_loads(self):
        import bass_rust as _bass_rust
        from concourse.hw_specs import get_activation_tables

        has_activation = any(
            isinstance(i, mybir.InstActivation)
            for b in self.main_func.blocks
            for i in b.instructions
        )
        if not has_activation:
            return
        tables = [
            (name, funcs if name == "natural_log_exp_and_others" else set())
            for name, funcs in get_activation_tables(self.m.arch).items()
        ]
        _bass_rust.insert_act_table_loads(self, tables)


def _build_program():
    nc = _Bacc("TRN2", target_bir_lowering=False, debug=False, num_devices=NC)

    xT_d = nc.dram_tensor("xT", [E, S], dt.float32r, kind="ExternalInput")
    wqkT_d = nc.dram_tensor("wqkT", [E, FQK], dt.float32r, kind="ExternalInput")
    wvT_d = nc.dram_tensor("wvT", [E, FV], dt.float32r, kind="ExternalInput")
    bqk_d = nc.dram_tensor("bqk", [FQK], dt.float32, kind="ExternalInput")
    bv_d = nc.dram_tensor("bv", [FV], dt.float32, kind="ExternalInput")
    wo_d = nc.dram_tensor("wo", [FV, E], dt.float32r, kind="ExternalInput")
    mask_d = nc.dram_tensor("trimask", [128, 128], dt.float32, kind="ExternalInput")
    sel_d = nc.dram_tensor("sel", [128, 128], dt.float32r, kind="ExternalInput")
    y_d = nc.dram_tensor("y", [S, E], dt.float32, kind="ExternalOutput")

    with TileKernel(nc) as tk:
        tk.build(xT_d, wqkT_d, wvT_d, bqk_d, bv_d, wo_d, mask_d, sel_d, y_d)
    nc.compile()
    return nc


class TileKernel:
    def __init__(self, nc):
        self.nc = nc
        self.ctx = ExitStack()
        self.tc_cm = tile.TileContext(nc)

    def __enter__(self):
        self.tc = self.tc_cm.__enter__()
        return self

    def __exit__(self, *a):
        self.ctx.close()
        return self.tc_cm.__exit__(*a)

    def build(self, xT_d, wqkT_d, wvT_d, bqk_d, bv_d, wo_d, mask_d, sel_d, y_d):
        nc, tc, ctx = self.nc, self.tc, self.ctx
        pool = lambda name, bufs, **kw: ctx.enter_context(
            tc.tile_pool(name=name, bufs=bufs, **kw)
        )

        const_p = pool("const", 1)
        xs_p = pool("xs", 2)
        qkt_p = pool("qkt", 1)
        vones_p = pool("vones", 1)
        attn_p = pool("attn", 8)
        pair_p = pool("pair", 1)
        small_p = pool("small", 2)
        y_p = pool("y", 4)
        # PSUM: ps (2 bufs x [128,1024] = 4 banks) + po (4 tags x 1 bank) = 8
        ps_p = pool("ps", 2, space="PSUM")
        po_p = pool("po", 1, space="PSUM")
        self.small_p = small_p
        self.ps_p = ps_p

        # ---- warm the exp activation table before any DMA lands ----
        warm = const_p.tile([1, 16], dt.float32, tag="warm")
        nc.vector.memset(warm[:], 0.0)
        nc.scalar.activation(warm[:], warm[:], mybir.ActivationFunctionType.Exp)

        # selector constant first: it is tiny and the PE warm-up dummies
        # depend on it
        sel_sb = const_p.tile([128, 128], dt.float32r, tag="sel")
        nc.sync.dma_start(sel_sb[:], sel_d[:])
        self.sel_sb = sel_sb

        # ---- weights/x: the first qkv matmul needs x0[ke] + wqk[ke], so
        # those DMAs are issued first, interleaved per-ke; everything else
        # queues behind them on the sync ring ----
        wqk_big = const_p.tile([128, KE * FQK], dt.float32r, tag="wqk")
        xs0 = xs_p.tile([128, KE * 512], dt.float32r, tag="xs", name="xs0")
        for ke in range(KE):
            nc.sync.dma_start(
                wqk_big[:, FQK * ke : FQK * (ke + 1)],
                wqkT_d[128 * ke : 128 * (ke + 1), :],
            )
            nc.scalar.dma_start(
                xs0[:, 512 * ke : 512 * (ke + 1)],
                xT_d[128 * ke : 128 * (ke + 1), 0:512],
            )
        wqk_r = [wqk_big[:, FQK * ke : FQK * (ke + 1)] for ke in range(KE)]
        self.xs0 = xs0

        bqk_sb = const_p.tile([128, 4], dt.float32, tag="bqk")
        nc.sync.dma_start(bqk_sb[:], bqk_d[:].rearrange("(f p) -> p f", p=128))
        bv_sb = const_p.tile([128, 2], dt.float32, tag="bv")
        nc.sync.dma_start(bv_sb[:], bv_d[:].rearrange("(f p) -> p f", p=128))
        mask_sb = const_p.tile([128, 128], dt.float32, tag="mask")
        nc.sync.dma_start(mask_sb[:], mask_d[:])

        wv_big = const_p.tile([128, KE * FV], dt.float32r, tag="wv")
        for ke in range(KE):
            nc.sync.dma_start(
                wv_big[:, FV * ke : FV * (ke + 1)],
                wvT_d[128 * ke : 128 * (ke + 1), :],
            )
        wv_r = [wv_big[:, FV * ke : FV * (ke + 1)] for ke in range(KE)]

        wo_big = const_p.tile([128, 2 * E], dt.float32r, tag="wo")
        for kt in range(2):
            nc.sync.dma_start(
                wo_big[:, E * kt : E * (kt + 1)],
                wo_d[128 * kt : 128 * (kt + 1), :],
            )
        wo_r = [wo_big[:, E * kt : E * (kt + 1)] for kt in range(2)]

        # persistent reciprocal tile (f32r: feeds the selector matmul).
        # ones-init once so untouched rows stay 1 through every reciprocal;
        # rollouts rewrite only rows 0/32/64/96
        onef = const_p.tile([128, 512], dt.float32, tag="onef")
        nc.vector.memset(onef[:], 1.0)
        self.rc = const_p.tile([128, 512], dt.float32r, tag="rc")
        nc.vector.tensor_copy(self.rc[:], onef[:])

        # ---- persistent activations ----
        # qkt tiles: 0: q heads 0,1 | 1: q heads 2,3 | 2: k heads 0,1 | 3: k heads 2,3
        qkt = [qkt_p.tile([128, S], dt.float32r, tag=f"qkt{f}", name=f"qkt{f}") for f in range(4)]
        # vones[t]: [v h0 |1| v h1 |1| v h2 |1| v h3 |1] for token tile t
        vones = [vones_p.tile([128, 4 * 65], dt.float32r, tag=f"v{t}", name=f"v{t}") for t in range(NT)]
        # ones columns are written once here; v_unit only writes the v parts
        ones_sb = const_p.tile([128, 1, 1], dt.float32, tag="ones")
        nc.vector.memset(ones_sb[:], 1.0)
        for t in range(NT):
            v3 = vones[t][:].rearrange("p (g d) -> p g d", d=65)
            nc.vector.tensor_copy(v3[:, :, 64:65], ones_sb[:].to_broadcast((128, 4, 1)))
        # pair tiles: final normalized attn output, [head dims x 2, S]
        pairt = [pair_p.tile([128, S], dt.float32r, tag=f"pair{hp}", name=f"pair{hp}") for hp in range(2)]

        env = dict(
            xT_d=xT_d, wqk_r=wqk_r, wv_r=wv_r, bqk_sb=bqk_sb,
            xs_p=xs_p, ps_p=ps_p, po_p=po_p, attn_p=attn_p, small_p=small_p,
            qkt=qkt, vones=vones,
            pairt=pairt, bv_sb=bv_sb, mask_sb=mask_sb, wo_r=wo_r,
            y_p=y_p, y_d=y_d,
        )
        # startup: chunk-0 x + qkv emitted directly. While the first x/wqk
        # pieces stream from HBM, run dummy matmuls on the tiny selector
        # constant so the PE p-state ramps before the real work lands.
        self.x_dma(0, env)
        pwarm = ps_p.tile([128, 1024], dt.float32, tag="ps", name="pwarm")
        for _ in range(12):
            nc.tensor.matmul(pwarm[:, 0:128], sel_sb[:, 0:128], sel_sb[:, 0:128],
                             start=True, stop=True)
        for u in self.qkv_units(0, env):
            u()
        # filler schedule: qkv(c+1) weaves into chunk c; out-projections are
        # pushed late so chunk 3 (the longest, scalar-bound) keeps PE work:
        # oproj(0)->c1/hp1, oproj(1)->c3/hp0, oproj(2)->c3/hp1, oproj(3)->tail
        deferred = []  # normalize units of the previous chunk's rollout
        for c in range(NCH):
            if c + 1 < NCH:
                self.x_dma(c + 1, env)
                qkv = self.qkv_units(c + 1, env)
                qk_u, v_u = qkv[:4], qkv[4:]
            else:
                qk_u, v_u = [], []
            f1 = list(v_u)
            if c == 3:
                other = self.oproj_units(1, env)
                f1 += self.oproj_units(0, env) + self.oproj_units(2, env)
            else:
                other = qk_u
            # interleave the deferred normalize units with the PE-heavy
            # fillers so vector work spreads across the phase
            f0 = []
            d, o = list(deferred), list(other)
            while d or o:
                if d:
                    f0.append(d.pop(0))
                if o:
                    f0.append(o.pop(0))
            deferred = self.attention_chunk(c, env, f0, f1)
        for u in deferred:
            u()
        for u in self.oproj_units(NCH - 1, env, copy_eng="scalar"):
            u()

    # ------------------------------------------------------------------
    def x_dma(self, c, env):
        nc = self.nc
        cs = slice(512 * c, 512 * (c + 1))
        if c == 0:
            env[("xs", 0)] = self.xs0
            return
        xs = env["xs_p"].tile([128, KE * 512], dt.float32r, tag="xs", name=f"xs{c}")
        # one descriptor-gen on the sync sequencer instead of eight
        nc.sync.dma_start(
            xs[:].rearrange("p (ke f) -> p ke f", f=512),
            env["xT_d"][:, cs].rearrange("(ke p) f -> p ke f", p=128),
        )
        env[("xs", c)] = xs

    # ------------------------------------------------------------------
    def qkv_units(self, c, env):
        nc = self.nc
        cs = slice(512 * c, 512 * (c + 1))
        wqk_r, wv_r = env["wqk_r"], env["wv_r"]
        qkt, vones = env["qkt"], env["vones"]
        bqk_sb = env["bqk_sb"]
        ps_p = env["ps_p"]
        xs = env[("xs", c)]
        xr = [xs[:, 512 * ke : 512 * (ke + 1)] for ke in range(KE)]

        def qk_unit(f):
            pq = ps_p.tile([128, 1024], dt.float32, tag="ps", name="pq")
            for ke in range(KE):
                nc.tensor.matmul(
                    pq[:, 0:512], wqk_r[ke][:, 128 * f : 128 * (f + 1)], xr[ke],
                    start=(ke == 0), stop=(ke == KE - 1),
                )
            nc.vector.tensor_scalar_add(qkt[f][:, cs], pq[:, 0:512], bqk_sb[:, f : f + 1])

        def v_unit(t4):
            t = 4 * c + t4
            pv = ps_p.tile([128, 1024], dt.float32, tag="ps", name="pv")
            for ke in range(KE):
                nc.tensor.matmul(
                    pv[:, 0:FV],
                    xr[ke][:, 128 * t4 : 128 * (t4 + 1)], wv_r[ke],
                    start=(ke == 0), stop=(ke == KE - 1),
                )
            v3 = vones[t][:].rearrange("p (g d) -> p g d", d=65)
            nc.vector.tensor_copy(
                v3[:, :, 0:64],
                pv[:, 0:FV].rearrange("p (g d) -> p g d", d=64),
            )

        units = []
        for f in range(4):
            units.append(lambda f=f: qk_unit(f))
        for t4 in range(4):
            units.append(lambda t4=t4: v_unit(t4))
        return units

    # ------------------------------------------------------------------
    def oproj_units(self, c, env, copy_eng="vector"):
        nc = self.nc
        pairt, wo_r, ps_p, y_p, y_d = (
            env["pairt"], env["wo_r"], env["ps_p"], env["y_p"], env["y_d"])
        units = []

        def unit(t4):
            t = 4 * c + t4
            ysb = y_p.tile([128, E], dt.float32, tag="y", name="ysb")
            py = ps_p.tile([128, 1024], dt.float32, tag="ps", name="py")
            for o in range(2):
                for kt in range(2):
                    nc.tensor.matmul(
                        py[:, 512 * o : 512 * (o + 1)],
                        pairt[kt][:, 128 * t : 128 * (t + 1)],
                        wo_r[kt][:, 512 * o : 512 * (o + 1)],
                        start=(kt == 0), stop=(kt == 1),
                    )
            if copy_eng == "scalar":
                nc.scalar.activation(ysb[:], py[:], mybir.ActivationFunctionType.Copy)
            else:
                nc.vector.tensor_copy(ysb[:], py[:])
            eng = nc.gpsimd if t % 2 == 0 else nc.sync
            eng.dma_start(y_d[128 * t : 128 * (t + 1), :], ysb[:])

        for t4 in range(4):
            units.append(lambda t4=t4: unit(t4))
        return units

    # ------------------------------------------------------------------
    def attention_chunk(self, c, env, fillers0, f1_extra):
        """Attention for both head pairs of chunk c. Returns the deferred
        normalize units of the hp=1 rollout for the caller to weave into
        the next phase."""
        nc = self.nc
        qkt, vones = env["qkt"], env["vones"]
        ps_p, po_p, attn_p = env["ps_p"], env["po_p"], env["attn_p"]
        mask_sb = env["mask_sb"]
        nj = 4 * c + 4
        # po[2*hp + h_idx]: [65, 512] accumulator per head
        po = [po_p.tile([65, 512], dt.float32, tag=f"po{i}", name=f"po{i}")
              for i in range(4)]

        def emit_pv(hp, j, off, at):
            for h_idx in range(2):
                slot = 2 * hp + h_idx
                nc.tensor.matmul(
                    po[slot][:, off:512],
                    vones[j][:, 65 * slot : 65 * slot + 65],
                    at[:, 512 * h_idx + off : 512 * (h_idx + 1)],
                    start=(j == 0), stop=(j == nj - 1),
                    skip_group_check=True,
                )

        last = c == NCH - 1
        part2_hp0 = None
        for hp in range(2):
            if hp == 0:
                fillers = fillers0
            else:
                fillers = (part2_hp0 or []) + f1_extra
            nfill = len(fillers)
            emitted = 0
            pending = []
            for j in range(nj):
                ps = ps_p.tile([128, 1024], dt.float32, tag="ps", name="ps")
                at = attn_p.tile([128, 1024], dt.float32r, tag="attn", name="at")
                m = j - 4 * c
                off = 128 * m if 1 <= m <= 3 else 0
                for h_idx in range(2):
                    # h1 writes full width so the single-run exp below never
                    # reads psum bytes this j didn't produce
                    off_mm = {1: 128, 2: 256, 3: 256}.get(m, 0) if h_idx == 0 else 0
                    r0 = 64 * h_idx
                    nc.tensor.matmul(
                        ps[:, 512 * h_idx + off_mm : 512 * (h_idx + 1)],
                        qkt[2 + hp][r0 : r0 + 64, 128 * j : 128 * (j + 1)],
                        qkt[hp][r0 : r0 + 64, 512 * c + off_mm : 512 * (c + 1)],
                        start=True, stop=True,
                    )
                nc.scalar.activation(
                    at[:, off:1024], ps[:, off:1024],
                    mybir.ActivationFunctionType.Exp)
                if m >= 0:
                    # causal mask: zero the upper triangle of the diagonal
                    # block, both heads in one strided op
                    av = at[:].rearrange("p (h q) -> p h q", h=2)
                    dg = av[:, :, 128 * m : 128 * (m + 1)]
                    mv = (mask_sb[:].rearrange("a (o n) -> a o n", o=1)
                          .to_broadcast((128, 2, 128)))
                    nc.vector.tensor_mul(dg, dg, mv)
                pending.append((j, off, at))
                if len(pending) > 6:
                    emit_pv(hp, *pending.pop(0))
                while emitted < nfill and emitted * nj < (j + 1) * nfill:
                    fillers[emitted]()
                    emitted += 1
            for p in pending:
                emit_pv(hp, *p)
            while emitted < nfill:
                fillers[emitted]()
                emitted += 1
            self.d_copies(hp, po)
            if last and hp == 0:
                part2_hp0 = self.rollout(c, env, po, [0])
        if last:
            return self.rollout(c, env, po, [1])
        return self.rollout(c, env, po, [0, 1])

    # ------------------------------------------------------------------
    def d_copies(self, hp, po):
        """Stage ln(denominator) for head pair hp into the reciprocal tile
        right after its last PV closes. On the scalar engine: it is idle at
        phase boundaries while vector is the critical queue, and staging the
        log makes 1/d a single Exp(-x) instead of a DVE reciprocal."""
        nc, rc = self.nc, self.rc
        ln = mybir.ActivationFunctionType.Ln
        nc.scalar.activation(rc[64 * hp : 64 * hp + 1, :], po[2 * hp][64:65, :], ln)
        nc.scalar.activation(rc[64 * hp + 32 : 64 * hp + 33, :], po[2 * hp + 1][64:65, :], ln)

    def rollout(self, c, env, po, hps):
        """Normalize head pairs `hps` of chunk c: pairt[hp][:,chunk] =
        po_v / denominator + bv. The reciprocal (split in halves so the
        vector queue stays granular) runs at the boundary; the broadcast
        (PE outer product with the selector into po's upper partitions),
        psum->sbuf staging, multiplies and bias-add are returned as
        deferred units the caller weaves into a later phase."""
        nc = self.nc
        small_p, pairt, bv_sb = self.small_p, env["pairt"], env["bv_sb"]
        rc = self.rc
        lo, hi = 64 * hps[0], 64 * hps[-1] + 33
        # 1/d = exp(-ln d); the Ln was fused into the denominator staging
        nc.scalar.activation(
            rc[lo:hi, :], rc[lo:hi, :], mybir.ActivationFunctionType.Exp,
            scale=-1.0)

        bchs = {}

        def bcast_stage():
            bch_ps = self.ps_p.tile([128, 1024], dt.float32, tag="ps", name="bch_ps")
            for hp in hps:
                nc.tensor.matmul(
                    bch_ps[:, 512 * hp : 512 * (hp + 1)],
                    self.sel_sb[64 * hp : 64 * hp + 64, :],
                    rc[64 * hp : 64 * hp + 64, :],
                    start=True, stop=True,
                )
            bchs["t"] = small_p.tile([128, 1024], dt.float32, tag="bch", name="bchs")
            if len(hps) == 2:
                nc.vector.tensor_copy(bchs["t"][:], bch_ps[:])
            else:
                hp = hps[0]
                nc.vector.tensor_copy(
                    bchs["t"][:, 512 * hp : 512 * (hp + 1)],
                    bch_ps[:, 512 * hp : 512 * (hp + 1)])

        tmps = {}

        def mul(hp):
            bch = bchs["t"]
            tmp = small_p.tile([128, 512], dt.float32, tag=f"tmp{hp}", name="tmp")
            nc.vector.tensor_mul(
                tmp[0:64, :], po[2 * hp][0:64, :], bch[0:64, 512 * hp : 512 * (hp + 1)])
            nc.vector.tensor_mul(
                tmp[64:128, :], po[2 * hp + 1][0:64, :],
                bch[64:128, 512 * hp : 512 * (hp + 1)])
            tmps[hp] = tmp

        def add(hp):
            nc.vector.tensor_scalar_add(
                pairt[hp][:, 512 * c : 512 * (c + 1)], tmps[hp][:],
                bv_sb[:, hp : hp + 1]
            )

        units = [bcast_stage]
        for hp in hps:
            units.append(lambda hp=hp: mul(hp))
            units.append(lambda hp=hp: add(hp))
        return units


# ----------------------------------------------------------------------
# ----------------------------------------------------------------------
_PROGRAM = None


def _get_program():
    global _PROGRAM
    if _PROGRAM is None:
        _PROGRAM = _build_program()
    return _PROGRAM


def _make_in_maps(inputs, W_in, b_in, W_out, b_out):
    in_maps = []
    scale = 1.0 / np.sqrt(np.float32(HD))
    kr = np.arange(128)[:, None]
    qc = np.arange(128)[None, :]
    trimask = np.where(qc >= kr, 1.0, 0.0).astype(np.float32)
    sel = np.zeros((128, 128), dtype=np.float32)
    sel[0, 0:64] = 1.0
    sel[32, 64:128] = 1.0
    sel[64, 0:64] = 1.0
    sel[96, 64:128] = 1.0
    for core in range(NC):
        b, g = divmod(core, 4)
        r = slice(256 * g, 256 * (g + 1))
        wq = W_in[0:E][r] * scale
        wk = W_in[E : 2 * E][r]
        wv = W_in[2 * E : 3 * E][r]
        xT = np.ascontiguousarray(inputs[b].T.astype(np.float32))
        wqkT = np.ascontiguousarray(np.concatenate([wq, wk], axis=0).T)
        wvT = np.ascontiguousarray(wv.T)
        bqk = np.concatenate([b_in[0:E][r] * scale, b_in[E : 2 * E][r]])
        bv = np.ascontiguousarray(b_in[2 * E : 3 * E][r])
        wo = np.ascontiguousarray(W_out[:, r].T)
        in_maps.append(
            {
                "xT": xT,
                "wqkT": wqkT.astype(np.float32),
                "wvT": wvT.astype(np.float32),
                "bqk": bqk.astype(np.float32),
                "bv": bv.astype(np.float32),
                "wo": wo.astype(np.float32),
                "trimask": trimask,
                "sel": sel,
            }
        )
    return in_maps


def run_spmd(inputs, W_in, b_in, W_out, b_out, trace=False, **kw):
    nc = _get_program()
    in_maps = _make_in_maps(inputs, W_in, b_in, W_out, b_out)
    bkr = run_bass_kernel_spmd(nc, in_maps, list(range(NC)), trace=trace, **kw)
    parts = [bkr.results[i]["y"] for i in range(NC)]
    out = np.stack(
        [
            parts[0] + parts[1] + parts[2] + parts[3],
            parts[4] + parts[5] + parts[6] + parts[7],
        ]
    )
    out = out + b_out[None, None, :]
    return out.astype(np.float32), bkr


def kernel(inputs, W_in, b_in, W_out, b_out):
    out, _ = run_spmd(
        np.asarray(inputs, dtype=np.float32),
        np.asarray(W_in, dtype=np.float32),
        np.asarray(b_in, dtype=np.float32),
        np.asarray(W_out, dtype=np.float32),
        np.asarray(b_out, dtype=np.float32),
    )
    return out

